# revision 33
# baseline (speedup 1.0000x reference)
"""Trainium2 Bass kernel for nn_AttnModel (gnn_message_passing).

Sharding: data-parallel over graphs B=32 across 8 cores (4 graphs/core).
Only collective: AllReduce of global-layer-norm sum/sumsq of t = nf@W^T+b.

Algebra (per core; x0 = GLN(t) = alpha*(t-m)):
  x_l = a_l*x0 + B_l@V^T      (gated residual stays in span{x0, V-cols})
  x_p_l = relu(P0a_l + B_l@VU)   P0a_l = a_l*(x0@U), via P0a *= om each layer
  zx_l = g0a_l + B_l@vg          g0a_l = a_l*(gw1.x0)
  zh_l = w_l.w2v                 w_l = relu(A_l y_p)
  om = 1-z = sigmoid(-(zpre+gb));  B' = om*(B-w)+w;  P0a *= om; g0a *= om
  out = sigmoid(sum_k x_p_2 * (x_p_2 @ YPY)),  YPY_g = y_p_g^T y_p_g

Layout: "4-stacked" [128, 4096]: partition rows 32g..32g+31 = k (or j) dim
of graph g; free = node index within graph. blockdiag lhsT [128,128] f32r
gives a 4-graph-parallel K=32 matmul at 1 cyc/col.

Softmax j-packing: j 0..31 of the 4 graphs fill a [128, 4096] tile (h0);
the leftover j 32..39 x 4 graphs = 32 rows are COMPACTED into one
[128, 1024] tile EB with rows 32q+8g+j' holding node-quarter q, so the
exp/reduce_max cost for them is 1/4 of a full pass instead of a full one.

Node column permutation: nf is DMA'd with PAIRED node rows (2 rows = 512B
per descriptor, full DMA bus efficiency).  Column c of every [128,4096]
tile maps to node  n(c) = 256*(c//256) + 2*(c%128) + (c//128)%2  within
its graph; the inverse permutation is applied on the host after gather.

Other structure: ONE packed-parameter DMA (HWDGE issue is ~625ns per DMA,
serialized), stats/collective chain gets PSUM priority over the frag
path, alpha = rsqrt(var) is computed via a pre-collective local seed +
3 DVE Newton steps so no ACT table switch is needed after the collective
(2 early table loads total), the global-layer-norm affine is fused into
the PSUM->SBUF copies of P0/g0, PE transposes run in f32r (1.5 cyc/row),
and each layer's gating chunk-loop is interleaved with the NEXT layer's
softmax pass (final sigmoid interleaved with layer 1) so ACT/DVE/Pool/PE
stay busy continuously.
"""

import numpy as np

B_ALL, NPG = 32, 4096
F, C, K, J = 64, 128, 32, 40
EPS = 1e-5
NC_ = 8
GPC = B_ALL // NC_            # 4 graphs per core
NLOC = GPC * NPG              # 16384 nodes per core
Q = NPG                       # 4096
CH = 512
NTOT = float(B_ALL * NPG * C)

_CACHE = {}


def _build(debug=False, collective=True):
    import concourse.bass as bass
    import concourse.mybir as mybir
    import concourse.tile as tile
    import concourse.bacc as bacc

    f32 = mybir.dt.float32
    f32r = mybir.dt.float32r
    bf16 = mybir.dt.bfloat16
    AF = mybir.ActivationFunctionType
    ALU = mybir.AluOpType
    AX = mybir.AxisListType

    nc = bacc.Bacc("TRN2", target_bir_lowering=False, debug=False, num_devices=NC_)

    def din(name, shape):
        return nc.dram_tensor(name, list(shape), f32, kind="ExternalInput")

    nf_d = din("nf", (NLOC, F))
    fragf_d = din("fragf", (B_ALL * J, F))     # full frag (stats), replicated
    fragl_d = din("fragl", (GPC * J, F))       # local 4 graphs' frag rows
    # all small parameters packed into ONE [128, 262] tensor (single DMA):
    # cols 0:128 ident | 128:192 W | 192:193 b | 193:225 U | 225:257 V |
    # 257 q(rows 0:32) | 258 gw1 | 259 gw2 | 260 gb | 261 unused
    allc_d = din("allc", (128, 262))
    # Sel quarters [64, 4x128] + g-block mask [128,128] for the EB
    # stationaries (engine partition starts must be 32-aligned, so the
    # [8,32] blocks are built via PE selector matmuls + a mask instead)
    allc2_d = din("allc2", (128, 640))
    out_d = nc.dram_tensor("out", [GPC, NPG], f32, kind="ExternalOutput")
    dbg = {}
    def dout(name, shape):
        if debug:
            dbg[name] = nc.dram_tensor("dbg_" + name, list(shape), f32,
                                       kind="ExternalOutput")
        return dbg.get(name)

    NFR = B_ALL * J     # 1280
    NFL = GPC * J       # 160
    NBF = NFR // 128    # 10 row-blocks of 128

    with tile.TileContext(nc) as tc:
        with (
            tc.tile_pool(name="const", bufs=1) as cst,
            tc.tile_pool(name="big", bufs=1) as big,
            tc.tile_pool(name="rot", bufs=2) as rot,
            tc.tile_pool(name="psb", bufs=2, space="PSUM") as psb,   # 2x[128,1024]
            tc.tile_pool(name="psc", bufs=2, space="PSUM") as psc,   # 2x[128,512]
            tc.tile_pool(name="psf", bufs=1, space="PSUM") as psf,   # frag path
            tc.tile_pool(name="psg", bufs=1, space="PSUM") as psg,   # Gram
            tc.tile_pool(name="dram", bufs=1, space="DRAM") as drp,
        ):
            def ctile(name, shape, dtype=f32):
                return cst.tile(list(shape), dtype, tag=name, name=name)

            def btile(name, shape, slot, dtype=f32):
                return big.tile(list(shape), dtype, tag=slot, name=name)

            def rtile(name, tag, dtype=f32):
                # chunk-local [128,1024] tiles, double-buffered
                return rot.tile([128, 1024], dtype, tag=tag, name=name)

            def pchunk(name="pch"):
                return psc.tile([128, CH], f32, tag="pch", name=name)

            def pbig(name="pI"):
                return psb.tile([128, 1024], f32, tag="pI", name=name)

            def pfrag(name="pfr"):
                return psf.tile([128, CH], f32, tag="pfr", name=name)

            # ---------------- DMA issue order ----------------
            allc = ctile("allc", (128, 262))
            nc.sync.dma_start(allc[:], allc_d.ap())

            allc2 = ctile("allc2", (128, 640))
            nc.sync.dma_start(allc2[:], allc2_d.ap())
            fragN = ctile("fragN", (128, NBF * F))
            nc.sync.dma_start(
                fragN[:].rearrange("p (b f) -> p b f", b=NBF),
                fragf_d.ap().rearrange("(b p) f -> p b f", p=128))
            fragNl = ctile("fragNl", (128, 2 * F))
            nc.sync.dma_start(fragNl[0:128, 0:F], fragl_d.ap()[0:128, :])
            nc.sync.dma_start(fragNl[0:NFL - 128, F:2 * F],
                              fragl_d.ap()[128:NFL, :])

            # nf halves, node-PAIRED descriptors (512B each):
            #   half[p, 128*b + 64*e + f] = nf[off + 256*b + 2*p + e, f]
            halves = []
            for piece in range(2):
                half = btile("nfnat_%d" % piece, (128, Q),
                             "S2" if piece == 0 else "S8", f32r)
                off = piece * (NLOC // 2)
                for hh in range(2):
                    nc.sync.dma_start(
                        half[:, 2048 * hh:2048 * hh + 2048]
                        .rearrange("p (b e f) -> p b e f", b=16, e=2),
                        nf_d.ap()[off + 4096 * hh:off + 4096 * hh + 4096, :]
                        .bitcast(f32r)
                        .rearrange("(b p e) f -> p b e f", b=16, p=128, e=2))
                halves.append(half)

            # ---------------- unpack consts (DVE, under DMA) --------------
            tId = ctile("ident", (128, 128))
            nc.vector.tensor_copy(tId[:], allc[:, 0:128])
            tW = ctile("W", (C, F))
            nc.vector.tensor_copy(tW[:], allc[:, 128:192])
            tb = ctile("b", (C, 1))
            nc.vector.tensor_copy(tb[:], allc[:, 192:193])
            tU = ctile("U", (C, K))
            nc.vector.tensor_copy(tU[:], allc[:, 193:225])
            tV = ctile("V", (C, K))
            nc.vector.tensor_copy(tV[:], allc[:, 225:257])
            tq = ctile("q", (K, 1))
            nc.vector.tensor_copy(tq[:], allc[0:K, 257:258])
            tgw1 = ctile("gw1", (C, 1))
            nc.vector.tensor_copy(tgw1[:], allc[:, 258:259])
            tgw2 = ctile("gw2", (C, 1))
            nc.vector.tensor_copy(tgw2[:], allc[:, 259:260])
            tgb = ctile("gb", (128, 1))
            nc.vector.tensor_copy(tgb[:], allc[:, 260:261])
            tIdr = ctile("identr", (128, 128), f32r)
            nc.vector.tensor_copy(tIdr[:], tId[:])
            tOnesRow = ctile("onesrow", (1, 128))
            nc.vector.memset(tOnesRow[:], 1.0)
            tOnes128 = ctile("ones128", (128, 1))
            nc.vector.memset(tOnes128[:], 1.0)
            tNgbH = ctile("ngbh", (128, 1))
            nc.vector.tensor_scalar_mul(tNgbH[:], tgb[:], -0.5)
            tOnes128b = ctile("ones128b", (128, 1), bf16)
            nc.vector.memset(tOnes128b[:], 1.0)
            tZero = ctile("zerof", (128, 128))
            nc.vector.memset(tZero[:], 0.0)

            def zfill(t):
                nc.gpsimd.tensor_copy(t[:], tZero[0:t.shape[0], 0:t.shape[1]])

            tIdbd = ctile("idbd_r", (128, 128), f32r)
            zfill(tIdbd)
            for g in range(GPC):
                sl = slice(K * g, K * g + K)
                nc.vector.tensor_copy(tIdbd[sl, sl], tId[0:K, 0:K])
            tIdbdH = ctile("idbd_h", (128, 128), f32r)
            tIdbdQ = ctile("idbd_q", (128, 128), f32r)
            nc.vector.tensor_scalar_mul(tIdbdH[:], tIdbd[:], 0.5)
            nc.vector.tensor_scalar_mul(tIdbdQ[:], tIdbd[:], 0.25)
            tSumbd = ctile("sumbd", (128, 128), f32r)
            zfill(tSumbd)
            for g in range(GPC):
                sl = slice(K * g, K * g + K)
                nc.vector.tensor_scalar(tSumbd[sl, sl], tId[0:K, 0:K], 0.0, 1.0,
                                        ALU.mult, ALU.add)   # ones block

            # ---------------- derived weights (psc matmuls) ----------------
            rhsUg = ctile("rhsUg", (C, K + 2))
            nc.vector.tensor_copy(rhsUg[:, 0:K], tU[:])
            nc.vector.tensor_copy(rhsUg[:, K:K + 1], tgw1[:])
            nc.vector.tensor_copy(rhsUg[:, K + 1:K + 2], tgw2[:])
            pw = pchunk()
            nc.tensor.matmul(pw[0:F, 0:K + 2], tW[:], rhsUg[:], start=True, stop=True)
            tWUg = ctile("WUg", (F, K + 2))
            nc.vector.tensor_copy(tWUg[:], pw[0:F, 0:K + 2])
            pw = pchunk()
            nc.tensor.matmul(pw[0:K, 0:K + 2], tV[:], rhsUg[:], start=True, stop=True)
            tVUg = ctile("VUg", (K, K + 2))
            nc.vector.tensor_copy(tVUg[:], pw[0:K, 0:K + 2])
            pw = pchunk()
            nc.tensor.matmul(pw[0:1, 0:K + 2], tOnes128[:], rhsUg[:], start=True, stop=True)
            tColF = ctile("colF", (1, K + 2))          # [colU | sg1 | sg2] free
            nc.vector.tensor_copy(tColF[:], pw[0:1, 0:K + 2])
            pw = pchunk()
            nc.tensor.transpose(pw[0:K + 2, 0:1], tColF[:], tId[0:1, 0:1])
            tColP = ctile("colP", (K + 2, 1))
            nc.vector.tensor_copy(tColP[:], pw[0:K + 2, 0:1])
            tColUrep = ctile("colUrep", (128, 1))
            for g in range(GPC):
                nc.vector.tensor_copy(tColUrep[K * g:K * g + K, :], tColP[0:K, :])

            # blockdiag lhsT for P0/g0: rows 64*gl..+64 = features of graph
            # 2p+gl; cols K*g..+K = WU (or gw1 replicated) -> out partition K*g
            tM_WU = ctile("M_WU", (128, 128), f32r)
            tM_G1 = ctile("M_G1", (128, 128), f32r)
            zfill(tM_WU)
            zfill(tM_G1)
            for piece in range(2):
                for gl in (0, 1):
                    g = 2 * piece + gl
                    nc.vector.tensor_copy(tM_WU[F * gl:F * gl + F, K * g:K * g + K],
                                          tWUg[:, 0:K])
                    nc.vector.tensor_copy(tM_G1[F * gl:F * gl + F, K * g:K * g + K],
                                          tWUg[:, K:K + 1].broadcast_to([F, K]))

            tBdVUq = ctile("bdVUq", (128, 128), f32r)
            tBdVUn = ctile("bdVUn", (128, 128), f32r)
            tBdVUp = ctile("bdVUp", (128, 128), f32r)
            tBdVGn = ctile("bdVGn", (128, 128), f32r)
            tBdW2V = ctile("bdW2V", (128, 128), f32r)
            zfill(tBdVUq)
            zfill(tBdVUn)
            zfill(tBdVUp)
            zfill(tBdVGn)
            zfill(tBdW2V)
            for g in range(GPC):
                sl = slice(K * g, K * g + K)
                nc.vector.tensor_scalar_mul(tBdVUq[sl, sl], tVUg[:, 0:K], -0.25)
                nc.vector.tensor_scalar_mul(tBdVUn[sl, sl], tVUg[:, 0:K], -0.5)
                nc.vector.tensor_copy(tBdVUp[sl, sl], tVUg[:, 0:K])
                nc.vector.tensor_scalar_mul(
                    tBdVGn[sl, sl],
                    tVUg[:, K:K + 1].broadcast_to([K, K]), -0.5)
                nc.vector.tensor_copy(tBdW2V[sl, sl],
                                      tVUg[:, K + 1:K + 2].broadcast_to([K, K]))

            pw = pchunk()
            nc.tensor.transpose(pw[0:F, 0:C], tW[:], tId[:])
            tWT = ctile("WT", (F, C))
            nc.vector.tensor_copy(tWT[:], pw[0:F, 0:C])

            # ---------------- frag path (early, own PSUM pool) ------------
            fragT = ctile("fragT", (F, NFR))
            for c0 in range(0, NBF, 4):
                nb = min(4, NBF - c0)
                pf = pfrag()
                for t in range(nb):
                    b = c0 + t
                    nc.tensor.transpose(pf[0:F, 128 * t:128 * t + 128],
                                        fragN[:, F * b:F * b + F], tId[:])
                nc.scalar.activation(fragT[:, 128 * c0:128 * (c0 + nb)],
                                     pf[0:F, 0:128 * nb], AF.Identity)
            ysT = ctile("ysT", (C, NFR))
            for c0 in range(0, NFR, CH):
                w_ = min(CH, NFR - c0)
                pf = pfrag()
                nc.tensor.matmul(pf[:, 0:w_], tWT[:], fragT[:, c0:c0 + w_],
                                 start=True, stop=True)
                nc.scalar.activation(ysT[:, c0:c0 + w_], pf[:, 0:w_],
                                     AF.Identity, bias=tb[:], scale=1.0)
            fsums = ctile("fsums", (128, 5))
            nc.vector.reduce_sum(fsums[:, 0:1], ysT[:], axis=AX.X)
            for ci, c0 in enumerate(range(0, NFR, CH)):
                w_ = min(CH, NFR - c0)
                pf = pfrag()
                nc.scalar.activation(pf[:, 0:w_], ysT[:, c0:c0 + w_], AF.Square,
                                     accum_out=fsums[:, 2 + ci:3 + ci])
            nc.vector.tensor_add(fsums[:, 1:2], fsums[:, 2:3], fsums[:, 3:4])
            nc.vector.tensor_add(fsums[:, 1:2], fsums[:, 1:2], fsums[:, 4:5])
            pf = pfrag()
            nc.tensor.matmul(pf[0:2, 0:1], fsums[:, 0:2], tOnes128[:],
                             start=True, stop=True)
            fs2 = ctile("fs2", (2, 1))
            nc.vector.tensor_copy(fs2[:], pf[0:2, 0:1])
            pf2 = pfrag()
            nc.tensor.transpose(pf2[0:1, 0:2], fs2[:], tId[0:2, 0:2])
            tFS = ctile("fragstat", (1, 8))
            nc.vector.tensor_copy(tFS[:, 0:2], pf2[0:1, 0:2])
            nfr = float(C * NFR)
            nc.vector.tensor_scalar_mul(tFS[:, 2:4], tFS[:, 0:2], 1.0 / nfr)
            nc.vector.tensor_mul(tFS[:, 4:5], tFS[:, 2:3], tFS[:, 2:3])
            nc.vector.tensor_sub(tFS[:, 5:6], tFS[:, 3:4], tFS[:, 4:5])
            nc.vector.tensor_scalar_add(tFS[:, 5:6], tFS[:, 5:6], EPS)
            nc.scalar.activation(tFS[:, 6:7], tFS[:, 5:6], AF.Sqrt)
            nc.vector.reciprocal(tFS[:, 7:8], tFS[:, 6:7])                  # a2
            nc.vector.tensor_mul(tFS[:, 4:5], tFS[:, 7:8], tFS[:, 2:3])
            nc.vector.tensor_scalar_mul(tFS[:, 4:5], tFS[:, 4:5], -1.0)    # -a2*m2
            tA2c = ctile("a2c", (128, 2))
            pf = pfrag()
            nc.tensor.matmul(pf[0:128, 0:1], tOnesRow[:], tFS[:, 7:8],
                             start=True, stop=True)
            nc.tensor.matmul(pf[0:128, 1:2], tOnesRow[:], tFS[:, 4:5],
                             start=True, stop=True)
            nc.vector.tensor_copy(tA2c[:], pf[0:128, 0:2])

            # local frag -> normalized ys (f32r) -> y_p smalls
            fragTl = ctile("fragTl", (F, NFL))
            pf = pfrag()
            nc.tensor.transpose(pf[0:F, 0:128], fragNl[:, 0:F], tId[:])
            nc.tensor.transpose(pf[0:F, 128:NFL], fragNl[0:NFL - 128, F:2 * F],
                                tId[0:NFL - 128, 0:NFL - 128])
            nc.vector.tensor_copy(fragTl[:], pf[0:F, 0:NFL])
            ysTl = ctile("ysTl", (C, NFL))
            pf = pfrag()
            nc.tensor.matmul(pf[:, 0:NFL], tWT[:], fragTl[:], start=True, stop=True)
            nc.scalar.activation(ysTl[:], pf[:, 0:NFL], AF.Identity,
                                 bias=tb[:], scale=1.0)
            ysnl = ctile("ysnl", (C, NFL), f32r)
            nc.scalar.activation(ysnl[:], ysTl[:], AF.Identity,
                                 bias=tA2c[:, 1:2], scale=tA2c[:, 0:1])
            tVr = ctile("Vr", (C, K), f32r)
            nc.vector.tensor_copy(tVr[:], tV[:])
            ypT = ctile("ypT", (K, NFL))
            for g in range(GPC):
                pf = pfrag()
                nc.tensor.matmul(pf[0:K, 0:J], tVr[:], ysnl[:, J * g:J * g + J],
                                 start=True, stop=True)
                nc.scalar.activation(ypT[:, J * g:J * g + J], pf[0:K, 0:J],
                                     AF.Relu, scale=tq[:])

            # y_p-derived stationaries
            tBdYT0 = ctile("bdYT0", (128, 128), f32r)
            zfill(tBdYT0)
            for g in range(GPC):
                sl = slice(K * g, K * g + K)
                nc.vector.tensor_copy(tBdYT0[sl, sl], ypT[:, J * g:J * g + K])
            ynat = ctile("ynat", (2 * K, 128))
            nc.vector.memset(ynat[:], 0.0)
            for g in range(GPC):
                pf = pfrag()
                nc.tensor.transpose(pf[0:J, 0:K], ypT[:, J * g:J * g + J],
                                    tId[0:K, 0:K])
                nc.vector.tensor_copy(ynat[0:J, K * g:K * g + K], pf[0:J, 0:K])
            # h0 block-diag ynat source for per-chunk softmax stationaries
            bdYsrc0 = ctile("bdYsrc0", (128, 128), f32r)
            zfill(bdYsrc0)
            for g in range(GPC):
                sl = slice(K * g, K * g + K)
                nc.vector.tensor_copy(bdYsrc0[sl, sl], ynat[0:K, sl])
            # h1 COMPACT quarter stationaries: EB rows = 32q + 8g + j'
            #  pI_B:   tBdYT1q[q][32g+k, 32q+8g+j'] = y_p[g, 32+j', k]
            #  w-side: bdYBsrc[q][32q+8g+j', 32g+k] = y_p[g, 32+j', k]
            tBdYT1q = [ctile("bdYT1q%d" % qq, (128, 128), f32r)
                       for qq in range(4)]
            bdYBsrc = [ctile("bdYBsrc%d" % qq, (128, 128), f32r)
                       for qq in range(4)]
            for qq in range(4):
                zfill(tBdYT1q[qq])
            for qq in range(4):
                for g in range(GPC):
                    r0 = 32 * qq + 8 * g
                    nc.vector.tensor_copy(
                        tBdYT1q[qq][K * g:K * g + K, r0:r0 + 8],
                        ypT[:, J * g + K:J * g + J])
            # bdYBsrc[q][32q+8g+j', 32g+k] = y_p[g,32+j',k]:
            # PE selector (rows <- ynat[K+j']) then mask off other g-blocks
            for qq in range(4):
                pf = pchunk()
                nc.tensor.matmul(pf[0:128, 0:128], allc2[0:2 * K, 128 * qq:128 * qq + 128],
                                 ynat[:], start=True, stop=True)
                nc.vector.tensor_mul(bdYBsrc[qq][:], pf[0:128, 0:128],
                                     allc2[:, 512:640])
            tBdYPY = ctile("bdYPY", (128, 128), f32r)
            zfill(tBdYPY)
            for g in range(GPC):
                pf = pfrag()
                nc.tensor.matmul(pf[0:K, 0:K], ynat[:, K * g:K * g + K],
                                 ynat[:, K * g:K * g + K], start=True, stop=True)
                nc.vector.tensor_copy(tBdYPY[K * g:K * g + K, K * g:K * g + K],
                                      pf[0:K, 0:K])

            # ------------ Gram stats (early collective) ------------
            psG = psg.tile([F, F + 1], f32, tag="psG", name="psG")

            halfbf = [btile("halfbf_0", (128, Q), "S9", bf16),
                      btile("halfbf_1", (128, Q), "S6", bf16)]

            def emit_cast(piece, hh):
                nc.vector.tensor_copy(
                    halfbf[piece][:, 2048 * hh:2048 * hh + 2048],
                    halves[piece][:, 2048 * hh:2048 * hh + 2048])

            def emit_gram(piece, hh):
                hbf = halfbf[piece]
                for b in range(32 * hh, 32 * hh + 32):
                    st = (piece == 0 and b == 0)
                    sp = (piece == 1 and b == 63)
                    tile_b = hbf[:, F * b:F * b + F]
                    nc.tensor.matmul(psG[:, 0:F], tile_b, tile_b,
                                     start=st, stop=sp, skip_group_check=True)
                    nc.tensor.matmul(psG[:, F:F + 1], tile_b, tOnes128b[:],
                                     start=st, stop=sp, skip_group_check=True)

            nfTs = [btile("nfT2_0", (128, Q), "S1", f32r),
                    btile("nfT2_1", (128, Q), "S3", f32r)]

            TENG = {0: "act", 1: "dve", 2: "act", 3: "dve",
                    4: "act", 5: "dve", 6: "act", 7: "dve"}

            def emit_transposes(piece, cc0, cc1):
                # cc indexes groups of 8 node-blocks -> one [64,1024] copy.
                # f32r transposes: 1.5 cyc/row on PE.
                half = halves[piece]
                nfT2 = nfTs[piece]
                for cc in range(cc0, cc1):
                    pf = pbig("pT_pI")
                    gl = cc // 4
                    for t in range(8):
                        b = 8 * cc + t
                        nc.tensor.transpose(
                            pf[0:F, 128 * t:128 * t + 128].bitcast(f32r),
                            half[:, F * b:F * b + F],
                            tIdr[:])
                    colb = (8 * cc % 32) * 128
                    dst = nfT2[F * gl:F * gl + F, colb:colb + 1024]
                    eng = TENG[cc]
                    if eng == "act":
                        nc.scalar.activation(dst, pf[0:F, 0:1024], AF.Identity)
                    elif eng == "dve":
                        nc.vector.tensor_copy(dst, pf[0:F, 0:1024])
                    else:
                        nc.gpsimd.tensor_copy(dst, pf[0:F, 0:1024])

            P0a = btile("P0a", (128, Q), "S4", f32r)
            g0a = btile("g0a", (128, Q), "S5", f32r)

            def emit_p0_chunk(piece, j2, fused):
                # P0 in one pbig; G0 in two psc chunks.  For piece 1 (post-
                # alpha) the global-layer-norm affine is FUSED into the
                # PSUM->SBUF copies; piece 0 runs before alpha is known, so
                # it copies raw (normalized later) to keep PSUM flowing.
                nfT2 = nfTs[piece]
                rsl = slice(64 * piece, 64 * piece + 64)
                pP = pbig("pP0")
                pGs = []
                for s in range(2):
                    cols = slice(1024 * j2 + CH * s, 1024 * j2 + CH * s + CH)
                    nc.tensor.matmul(pP[:, CH * s:CH * s + CH], tM_WU[:],
                                     nfT2[:, cols], start=True, stop=True)
                    pG = pchunk("pG0")
                    nc.tensor.matmul(pG[:], tM_G1[:],
                                     nfT2[:, cols], start=True, stop=True)
                    pGs.append(pG)
                cols2 = slice(1024 * j2, 1024 * j2 + 1024)
                pe = ("act", "dve", "dve", "act")[j2]
                ge = (("dve", "act"), ("act", "dve"),
                      ("dve", "act"), ("act", "dve"))[j2]
                if fused:
                    psc_, pbi = (tAB[rsl, 0:1], tBiasP0[rsl, :])
                    gsc_, gbi = (tAB[rsl, 0:1], tAB[rsl, 2:3])
                if pe == "act":
                    if fused:
                        nc.scalar.activation(P0a[rsl, cols2], pP[rsl, :],
                                             AF.Identity, bias=pbi, scale=psc_)
                    else:
                        nc.scalar.activation(P0a[rsl, cols2], pP[rsl, :],
                                             AF.Identity)
                elif pe == "dve":
                    if fused:
                        nc.vector.tensor_scalar(P0a[rsl, cols2], pP[rsl, :],
                                                psc_, pbi, ALU.mult, ALU.add)
                    else:
                        nc.vector.tensor_copy(P0a[rsl, cols2], pP[rsl, :])
                else:
                    if fused:
                        nc.gpsimd.tensor_scalar(P0a[rsl, cols2], pP[rsl, :],
                                                psc_, pbi, ALU.mult, ALU.add)
                    else:
                        nc.gpsimd.tensor_copy(P0a[rsl, cols2], pP[rsl, :])
                for s in range(2):
                    colh = slice(1024 * j2 + CH * s, 1024 * j2 + CH * s + CH)
                    eng = ge[s]
                    if eng == "act":
                        if fused:
                            nc.scalar.activation(g0a[rsl, colh], pGs[s][rsl, :],
                                                 AF.Identity, bias=gbi,
                                                 scale=gsc_)
                        else:
                            nc.scalar.activation(g0a[rsl, colh], pGs[s][rsl, :],
                                                 AF.Identity)
                    elif eng == "dve":
                        if fused:
                            nc.vector.tensor_scalar(g0a[rsl, colh], pGs[s][rsl, :],
                                                    gsc_, gbi, ALU.mult, ALU.add)
                        else:
                            nc.vector.tensor_copy(g0a[rsl, colh], pGs[s][rsl, :])
                    else:
                        if fused:
                            nc.gpsimd.tensor_scalar(g0a[rsl, colh], pGs[s][rsl, :],
                                                    gsc_, gbi, ALU.mult, ALU.add)
                        else:
                            nc.gpsimd.tensor_copy(g0a[rsl, colh], pGs[s][rsl, :])

            # PE emission: casts/grams first so the stats chain (-> the
            # AllReduce) issues as early as the nf DMA allows.
            emit_cast(0, 0)
            emit_cast(0, 1)
            emit_gram(0, 0)
            emit_gram(0, 1)
            emit_transposes(0, 0, 4)
            emit_cast(1, 0)
            emit_cast(1, 1)
            emit_gram(1, 0)
            emit_gram(1, 1)

            # ---- local stats -> AllReduce ----
            tGs = ctile("Gs", (F, F + 1))
            nc.vector.tensor_copy(tGs[:], psG[:])
            pf = pchunk()
            nc.tensor.matmul(pf[0:F, 0:C], tGs[:, 0:F], tWT[:], start=True, stop=True)
            tGW = ctile("GW", (F, C))
            nc.vector.tensor_mul(tGW[:], pf[0:F, 0:C], tWT[:])
            pf = pchunk()
            nc.tensor.matmul(pf[0:C, 0:1], tGW[:], tOnes128[0:F, :],
                             start=True, stop=True)              # quad_c
            nc.tensor.matmul(pf[0:C, 1:2], tWT[:], tGs[:, F:F + 1],
                             start=True, stop=True)              # ws_c
            tM5 = ctile("M5", (128, 5))
            nc.vector.tensor_copy(tM5[:, 0:2], pf[0:C, 0:2])
            nc.vector.tensor_copy(tM5[:, 2:3], tb[:])
            nc.vector.tensor_mul(tM5[:, 3:4], tb[:], tb[:])
            nc.vector.tensor_mul(tM5[:, 4:5], tb[:], tM5[:, 1:2])
            pf = pchunk()
            nc.tensor.matmul(pf[0:5, 0:1], tM5[:], tOnes128[:], start=True, stop=True)
            st5 = ctile("st5", (5, 1))
            nc.vector.tensor_copy(st5[:], pf[0:5, 0:1])
            pf2 = pchunk()
            nc.tensor.transpose(pf2[0:1, 0:5], st5[:], tId[0:5, 0:5])
            tST = ctile("stat", (1, 12))
            nc.vector.tensor_copy(tST[:, 0:5], pf2[0:1, 0:5])
            # [0]=quad [1]=ws [2]=b [3]=b2 [4]=bws
            nc.vector.tensor_scalar(tST[:, 5:6], tST[:, 2:3], float(NLOC), None,
                                    ALU.mult)
            nc.vector.tensor_add(tST[:, 5:6], tST[:, 5:6], tST[:, 1:2])
            nc.vector.tensor_scalar(tST[:, 6:7], tST[:, 4:5], 2.0, None, ALU.mult)
            nc.vector.tensor_add(tST[:, 6:7], tST[:, 6:7], tST[:, 0:1])
            nc.vector.tensor_scalar(tST[:, 7:8], tST[:, 3:4], float(NLOC), None,
                                    ALU.mult)
            nc.vector.tensor_add(tST[:, 6:7], tST[:, 6:7], tST[:, 7:8])

            # local rsqrt(var) seed: the global alpha is then refined with
            # three DVE Newton steps, so no ACT Sqrt is needed after the
            # collective (exp table set stays resident from the warm on).
            tLoc = ctile("locstat", (1, 6))
            nc.vector.tensor_scalar_mul(tLoc[:, 0:2], tST[:, 5:7],
                                        1.0 / (float(NLOC) * C))
            nc.vector.tensor_mul(tLoc[:, 2:3], tLoc[:, 0:1], tLoc[:, 0:1])
            nc.vector.tensor_sub(tLoc[:, 2:3], tLoc[:, 1:2], tLoc[:, 2:3])
            nc.vector.tensor_scalar_add(tLoc[:, 2:3], tLoc[:, 2:3], EPS)
            nc.scalar.activation(tLoc[:, 3:4], tLoc[:, 2:3], AF.Sqrt)
            nc.vector.reciprocal(tLoc[:, 4:5], tLoc[:, 3:4])     # y0 seed

            cin = ctile("cin", (1, 128))
            nc.vector.memset(cin[:], 0.0)
            nc.vector.tensor_copy(cin[:, 0:1], tST[:, 5:6])
            nc.vector.tensor_copy(cin[:, 1:2], tST[:, 6:7])
            db_in = drp.tile([1, 128], f32, name="db_in")
            db_out = drp.tile([1, 128], f32, name="db_out")
            nc.sync.dma_start(db_in[:], cin[:])
            if collective:
                nc.gpsimd.collective_compute(
                    "AllReduce", mybir.AluOpType.add,
                    replica_groups=[list(range(NC_))],
                    ins=[db_in.opt()], outs=[db_out.opt()],
                )
            else:
                nc.sync.dma_start(db_out[:], db_in[:])
            cout = ctile("cout", (1, 128))
            nc.sync.dma_start(cout[:], db_out[:])

            # swap the ACT table to the exp/tanh set; depends on BOTH early
            # sqrt outputs so the scheduler cannot hoist it before them.
            tWarm = ctile("warm", (1, 2))
            nc.vector.tensor_add(tWarm[:, 0:1], tLoc[:, 3:4], tFS[:, 6:7])
            nc.scalar.activation(tWarm[:, 1:2], tWarm[:, 0:1], AF.Exp)

            # ---- transposes + P0/g0; piece 0 raw while alpha is in flight
            emit_transposes(0, 4, 8)
            for _j2 in range(4):
                emit_p0_chunk(0, _j2, fused=False)
            emit_transposes(1, 0, 8)

            # ---- alpha/bias from the AllReduce result (DVE only) ----
            tGS = ctile("gstat", (1, 8))
            nc.vector.tensor_scalar_mul(tGS[:, 0:2], cout[:, 0:2], 1.0 / NTOT)
            nc.vector.tensor_mul(tGS[:, 2:3], tGS[:, 0:1], tGS[:, 0:1])
            nc.vector.tensor_sub(tGS[:, 2:3], tGS[:, 1:2], tGS[:, 2:3])
            nc.vector.tensor_scalar_add(tGS[:, 2:3], tGS[:, 2:3], EPS)
            # alpha = rsqrt(v) by Newton from the local seed:
            # y <- y*(1.5 - 0.5*v*y^2), three times (on Pool: its queue is
            # quiet when the AllReduce lands, DVE's is not)
            tNR = ctile("newton", (1, 4))
            nc.vector.tensor_copy(tNR[:, 0:1], tLoc[:, 4:5])
            for _ in range(3):
                nc.vector.tensor_mul(tNR[:, 1:2], tNR[:, 0:1], tNR[:, 0:1])
                nc.vector.tensor_mul(tNR[:, 1:2], tNR[:, 1:2], tGS[:, 2:3])
                nc.vector.tensor_scalar(tNR[:, 2:3], tNR[:, 1:2], -0.5, 1.5,
                                        ALU.mult, ALU.add)
                nc.vector.tensor_mul(tNR[:, 0:1], tNR[:, 2:3], tNR[:, 0:1])
            nc.vector.tensor_copy(tGS[:, 4:5], tNR[:, 0:1])            # alpha
            nc.vector.tensor_mul(tGS[:, 5:6], tGS[:, 4:5], tGS[:, 0:1])
            nc.vector.tensor_scalar_mul(tGS[:, 5:6], tGS[:, 5:6], -1.0)  # -am
            nc.vector.tensor_mul(tGS[:, 6:7], tGS[:, 5:6], tColF[:, K:K + 1])
            tAB = ctile("alphab", (128, 3))
            pf = pchunk()
            for ii, cidx in [(0, 4), (1, 5), (2, 6)]:
                nc.tensor.matmul(pf[0:128, ii:ii + 1], tOnesRow[:],
                                 tGS[:, cidx:cidx + 1], start=True, stop=True)
            nc.vector.tensor_copy(tAB[:], pf[0:128, 0:3])
            tBiasP0 = ctile("biasP0", (128, 1))
            nc.vector.tensor_mul(tBiasP0[:], tColUrep[:], tAB[:, 1:2])

            # ---- piece-1 P0/g0 fused-normalized; piece-0 rows normalized
            # per column chunk so xp0/E0 can start chunk-by-chunk
            for _j2 in range(4):
                emit_p0_chunk(1, _j2, fused=True)
                nc0 = slice(1024 * _j2, 1024 * _j2 + 1024)
                nc.gpsimd.tensor_scalar(P0a[0:64, nc0], P0a[0:64, nc0],
                                        tAB[0:64, 0:1], tBiasP0[0:64, :],
                                        ALU.mult, ALU.add)
                nc.vector.tensor_scalar(g0a[0:64, nc0], g0a[0:64, nc0],
                                        tAB[0:64, 0:1], tAB[0:64, 2:3],
                                        ALU.mult, ALU.add)
            if debug:
                d = dout("P0a", (128, Q)); nc.sync.dma_start(d.ap(), P0a[:].bitcast(f32))
                d = dout("g0a", (128, Q)); nc.sync.dma_start(d.ap(), g0a[:].bitcast(f32))
                d = dout("gstat", (1, 7)); nc.sync.dma_start(d.ap(), tGS[:, 0:7])

            # =================== softmax + gating layers ===================
            # E-state tiles per layer
            Etiles, EBtiles, nmxs, Ss, Bsts, fscs = {}, {}, {}, {}, {}, {}
            bdY0c = [ctile("bdY0c_%d" % c, (128, 128), f32r) for c in range(4)]
            bdYBc = [ctile("bdYBc_%d" % c, (128, 128), f32r) for c in range(4)]
            xps = {}

            def emit_E_h0_chunk(l, cq, xp):
                E, nmx, S = Etiles[l], nmxs[l], Ss[l]
                pI = pbig("pI")
                for s2 in range(2):
                    cs = slice(1024 * cq + CH * s2, 1024 * cq + CH * s2 + CH)
                    nc.tensor.matmul(pI[:, CH * s2:CH * s2 + CH],
                                     tBdYT0[:], xp[:, cs],
                                     start=True, stop=True)
                nc.vector.reduce_max(nmx[:, cq:cq + 1], pI[:], axis=AX.X,
                                     negate=True)
                nc.scalar.activation(E[:, 1024 * cq:1024 * cq + 1024],
                                     pI[:], AF.Exp, bias=nmx[:, cq:cq + 1],
                                     accum_out=S[:, cq:cq + 1])

            def emit_E_B(l, xp):
                # compact h1 tile: rows 32q+8g+j' <- node-quarter q
                EB, nmx, S = EBtiles[l], nmxs[l], Ss[l]
                pIB = pbig("pIB")
                for s2 in range(2):
                    lc = slice(CH * s2, CH * s2 + CH)
                    for qq in range(4):
                        nc.tensor.matmul(pIB[:, lc], tBdYT1q[qq][:],
                                         xp[:, 1024 * qq + CH * s2:
                                             1024 * qq + CH * s2 + CH],
                                         start=(qq == 0), stop=(qq == 3))
                nc.vector.reduce_max(nmx[:, 4:5], pIB[:], axis=AX.X,
                                     negate=True)
                nc.scalar.activation(EB[:], pIB[:], AF.Exp,
                                     bias=nmx[:, 4:5], accum_out=S[:, 4:5])

            def emit_combines(l):
                nmx, S, fsc = nmxs[l], Ss[l], fscs[l]
                # ---- h0: per-(g,j) over the 4 column chunks ----
                nc.vector.tensor_reduce(nmx[:, 5:6], nmx[:, 0:4], AX.X, ALU.min)
                nc.vector.tensor_scalar(fsc[:, 0:4], nmx[:, 0:4],
                                        nmx[:, 5:6], None, ALU.subtract)
                nc.scalar.activation(fsc[:, 0:4], fsc[:, 0:4], AF.Exp,
                                     scale=-1.0)
                nc.vector.tensor_mul(S[:, 0:4], S[:, 0:4], fsc[:, 0:4])
                nc.vector.tensor_reduce(S[:, 5:6], S[:, 0:4], AX.X, ALU.add)
                nc.vector.reciprocal(S[:, 6:7], S[:, 5:6])
                nc.vector.tensor_scalar(fsc[:, 0:4], fsc[:, 0:4],
                                        S[:, 6:7], None, ALU.mult)
                for c in range(4):
                    nc.gpsimd.tensor_scalar(bdY0c[c][:], bdYsrc0[:],
                                            fsc[:, c:c + 1], None, ALU.mult)
                # ---- EB: per-(g,j') over the 4 quarter ROW blocks ----
                # transpose nmxB and SB to the free axis (partition 0)
                pq = pchunk("pqB")
                nc.tensor.transpose(pq[0:1, 0:128], nmx[:, 4:5], tId[:])
                nc.tensor.transpose(pq[0:1, 128:256], S[:, 4:5], tId[:])
                tNB = ctile("tNB%d" % l, (1, 256 + 128))
                nc.vector.tensor_copy(tNB[:, 0:256], pq[0:1, 0:256])
                tGB = ctile("tGB%d" % l, (1, 96))
                # global (negated) max over quarters, per (g,j')
                nc.vector.tensor_reduce(
                    tGB[:, 0:32],
                    tNB[:, 0:128].rearrange("p (qq r) -> p r qq", qq=4),
                    AX.X, ALU.min)
                for qq in range(4):
                    nc.vector.tensor_sub(tNB[:, 256 + 32 * qq:256 + 32 * qq + 32],
                                         tNB[:, 32 * qq:32 * qq + 32],
                                         tGB[:, 0:32])
                nc.scalar.activation(tNB[:, 256:384], tNB[:, 256:384],
                                     AF.Exp, scale=-1.0)        # rescale f_q
                nc.vector.tensor_mul(tNB[:, 128:256], tNB[:, 128:256],
                                     tNB[:, 256:384])           # S_q * f_q
                nc.vector.tensor_reduce(
                    tGB[:, 32:64],
                    tNB[:, 128:256].rearrange("p (qq r) -> p r qq", qq=4),
                    AX.X, ALU.add)
                nc.vector.reciprocal(tGB[:, 64:96], tGB[:, 32:64])
                for qq in range(4):
                    nc.vector.tensor_mul(tNB[:, 256 + 32 * qq:256 + 32 * qq + 32],
                                         tNB[:, 256 + 32 * qq:256 + 32 * qq + 32],
                                         tGB[:, 64:96])
                pq2 = pchunk("pqB2")
                nc.tensor.transpose(pq2[0:128, 0:1], tNB[:, 256:384], tId[0:1, 0:1])
                tFscB = ctile("tFscB%d" % l, (128, 1))
                nc.vector.tensor_copy(tFscB[:], pq2[0:128, 0:1])
                for c in range(4):
                    nc.gpsimd.tensor_scalar(bdYBc[c][:], bdYBsrc[c][:],
                                            tFscB[:, 0:1], None, ALU.mult)

            def emit_layer_chunk(l, cc):
                E, EB = Etiles[l], EBtiles[l]
                Bst = Bsts.get(l - 1)
                xpn = xps[l + 1]
                cols = slice(1024 * cc, 1024 * cc + 1024)
                pW = pbig("pW")
                for s2 in range(2):
                    lc = slice(CH * s2, CH * s2 + CH)
                    cs = slice(1024 * cc + CH * s2, 1024 * cc + CH * s2 + CH)
                    nc.tensor.matmul(pW[:, lc], bdY0c[cc][:], E[:, cs],
                                     start=True, stop=False)
                    nc.tensor.matmul(pW[:, lc], bdYBc[cc][:], EB[:, lc],
                                     start=False, stop=True)
                wt = rtile("w%d_%d" % (l, cc), "wt", f32r)
                nc.scalar.activation(wt[:], pW[:], AF.Relu)
                pZ = pbig("pZ")
                for s2 in range(2):
                    lc = slice(CH * s2, CH * s2 + CH)
                    cs = slice(1024 * cc + CH * s2, 1024 * cc + CH * s2 + CH)
                    nc.tensor.matmul(pZ[:, lc], tBdW2V[:], wt[:, lc],
                                     start=True, stop=False)
                    if l == 1:
                        nc.tensor.matmul(pZ[:, lc], tBdVGn[:], Bst[:, cs],
                                         start=False, stop=False)
                    nc.tensor.matmul(pZ[:, lc], tIdbd if l == 0 else tIdbdH,
                                     g0a[:, cs], start=False, stop=True)
                # t = tanh(-(zs+gb)/2); om = 0.5+0.5t never materialized:
                # consumers fuse (t+-1), stationaries absorb the 2x
                om = rtile("om%d_%d" % (l, cc), "om")
                nc.scalar.activation(om[:], pZ[:], AF.Tanh,
                                     bias=tNgbH[:], scale=-0.5)
                if l == 0:
                    # 2*nB1 = (t - 1)*w ; P0a doubles: (t + 1)*P0a; g0a same
                    nc.vector.scalar_tensor_tensor(
                        Bsts[0][:, cols], om[:], 1.0, wt[:],
                        ALU.subtract, ALU.mult)
                    nc.vector.scalar_tensor_tensor(
                        P0a[:, cols], om[:], 1.0, P0a[:, cols],
                        ALU.add, ALU.mult)
                    nc.vector.scalar_tensor_tensor(
                        g0a[:, cols], om[:], 1.0, g0a[:, cols],
                        ALU.add, ALU.mult)
                    # x_p for layer 1 = relu(P0a + B@VU), 512-wide via psc
                    for s2 in range(2):
                        cs = slice(1024 * cc + CH * s2,
                                   1024 * cc + CH * s2 + CH)
                        pX = pchunk("pX")
                        nc.tensor.matmul(pX[:], tBdVUn[:], Bsts[0][:, cs],
                                         start=True, stop=False)
                        nc.tensor.matmul(pX[:], tIdbdH, P0a[:, cs],
                                         start=False, stop=True)
                        nc.vector.tensor_scalar(xpn[:, cs], pX[:], 0.0, None,
                                                ALU.max)
                else:
                    # layer-1 update via gate-matmul commutation: the gate
                    # (t+1)/2 is per-NODE (per column), so it commutes with
                    # the k-contractions:
                    #   xi_2 = (t1+1) (x) pA + pB
                    #   pA = P0a/4 + VU@B1/2 - VU@w1/2
                    #      = IdbdQ@P0a + (-VU/4)@Bst + (-VU/2)@w1
                    #   pB = VU@w1
                    # no md/md2/P0a stt passes at all in this layer.
                    pA = pbig("pA")
                    for s2 in range(2):
                        lc = slice(CH * s2, CH * s2 + CH)
                        cs = slice(1024 * cc + CH * s2,
                                   1024 * cc + CH * s2 + CH)
                        nc.tensor.matmul(pA[:, lc], tIdbdQ, P0a[:, cs],
                                         start=True, stop=False)
                        nc.tensor.matmul(pA[:, lc], tBdVUq[:], Bst[:, cs],
                                         start=False, stop=False)
                        nc.tensor.matmul(pA[:, lc], tBdVUn[:], wt[:, lc],
                                         start=False, stop=True)
                    c1 = rtile("c1_%d" % cc, "c1", f32r)
                    nc.vector.scalar_tensor_tensor(
                        c1[:], om[:], 1.0, pA[:], ALU.add, ALU.mult)
                    for s2 in range(2):
                        lc = slice(CH * s2, CH * s2 + CH)
                        cs = slice(1024 * cc + CH * s2,
                                   1024 * cc + CH * s2 + CH)
                        pB = pchunk("pB")
                        nc.tensor.matmul(pB[:], tBdVUp[:], wt[:, lc],
                                         start=True, stop=True)
                        nc.vector.scalar_tensor_tensor(
                            c1[:, lc], c1[:, lc], 0.0, pB[:],
                            ALU.add, ALU.add)
                        nc.vector.tensor_scalar(xpn[:, cs], c1[:, lc],
                                                0.0, None, ALU.max)

            def emit_final_chunk(j2, xp):
                cols2 = slice(1024 * j2, 1024 * j2 + 1024)
                pP = pbig("pF")
                for s in range(2):
                    cols = slice(1024 * j2 + CH * s, 1024 * j2 + CH * s + CH)
                    nc.tensor.matmul(pP[:, CH * s:CH * s + CH], tBdYPY[:],
                                     xp[:, cols], start=True, stop=True)
                tmp = rtile("tmp%d" % j2, "tmp", f32r)
                nc.vector.tensor_mul(tmp[:], pP[:], xp[:, cols2])
                pS = pbig("pS")
                for s in range(2):
                    lc = slice(CH * s, CH * s + CH)
                    nc.tensor.matmul(pS[:, lc], tSumbd[:], tmp[:, lc],
                                     start=True, stop=True)
                sO = rtile("sOut%d" % j2, "sOut")
                # host applies sigmoid = 0.5*tanh+0.5 after the gather
                nc.scalar.activation(sO[:], pS[:], AF.Tanh, scale=0.5)
                for g in range(GPC):
                    nc.sync.dma_start(out_d.ap()[g:g + 1, cols2],
                                      sO[K * g:K * g + 1, :])

            # ---- allocate per-layer E-state (in first-write order) ----
            xps[0] = btile("xp_0", (128, Q), "S6", f32r)
            Etiles[0] = btile("E0", (128, Q), "S1", f32r)
            EBtiles[0] = btile("EB0", (128, 1024), "S9", f32r)
            xps[1] = btile("xp_1", (128, Q), "S6", f32r)
            Bsts[0] = btile("B0", (128, Q), "S3", f32r)
            Etiles[1] = btile("E1", (128, Q), "S8", f32r)
            EBtiles[1] = btile("EB1", (128, 1024), "S9", f32r)
            xps[2] = btile("xp_2", (128, Q), "S6", f32r)
            for l in range(2):
                nmxs[l] = ctile("nmx%d" % l, (128, 6))
                Ss[l] = ctile("S%d" % l, (128, 7))
                fscs[l] = ctile("fsc%d" % l, (128, 4))

            # ---- xp0 (DVE relu from normalized P0a) interleaved with E0 ----
            xp0 = xps[0]
            for cq in range(4):
                xc = slice(1024 * cq, 1024 * cq + 1024)
                nc.vector.tensor_scalar(xp0[:, xc], P0a[:, xc], 0.0, None,
                                        ALU.max)
                if cq >= 1:
                    emit_E_h0_chunk(0, cq - 1, xp0)
            emit_E_h0_chunk(0, 3, xp0)
            emit_E_B(0, xp0)
            emit_combines(0)
            if debug:
                d = dout("xp0", (128, Q)); nc.sync.dma_start(d.ap(), xp0[:].bitcast(f32))

            # ---- layer 0 chunks interleaved with layer-1 softmax ----
            for cc in range(4):
                emit_layer_chunk(0, cc)
                if cc >= 2:
                    emit_E_h0_chunk(1, cc - 2, xps[1])
            emit_E_h0_chunk(1, 2, xps[1])
            emit_E_h0_chunk(1, 3, xps[1])
            emit_E_B(1, xps[1])
            emit_combines(1)
            if debug:
                d = dout("xp1", (128, Q)); nc.sync.dma_start(d.ap(), xps[1][:].bitcast(f32))

            # ---- layer 1 chunks interleaved with the final sigmoid ----
            for cc in range(4):
                emit_layer_chunk(1, cc)
                if cc >= 2:
                    emit_final_chunk(cc - 2, xps[2])
            emit_final_chunk(2, xps[2])
            emit_final_chunk(3, xps[2])
            if debug:
                d = dout("xp2", (128, Q)); nc.sync.dma_start(d.ap(), xps[2][:].bitcast(f32))


    nc.compile()
    return nc


def _get_program(debug=False):
    key = "nc_dbg" if debug else "nc"
    if key not in _CACHE:
        _CACHE[key] = _build(debug)
    return _CACHE[key]


def make_in_maps(inputs):
    nf = np.ascontiguousarray(np.asarray(inputs["node_feats"], np.float32))
    frag = np.ascontiguousarray(
        np.asarray(inputs["frag_emb"], np.float32).reshape(B_ALL * J, F))
    W = np.ascontiguousarray(np.asarray(inputs["W_in"], np.float32))
    b = np.asarray(inputs["b_in"], np.float32).reshape(C, 1)
    U = np.ascontiguousarray(np.asarray(inputs["U"], np.float32))
    V = np.ascontiguousarray(np.asarray(inputs["V"], np.float32))
    q = np.asarray(inputs["q"], np.float32).reshape(K, 1)
    gW = np.asarray(inputs["gate_W"], np.float32).reshape(2 * C)
    gb = np.asarray(inputs["gate_b"], np.float32).reshape(1)
    allc = np.zeros((128, 262), np.float32)
    allc[:, 0:128] = np.eye(128, dtype=np.float32)
    allc[:, 128:192] = W
    allc[:, 192:193] = b
    allc[:, 193:225] = U
    allc[:, 225:257] = V
    allc[0:K, 257] = q[:, 0]
    allc[:, 258] = gW[:C]
    allc[:, 259] = gW[C:]
    allc[:, 260] = gb[0]
    allc2 = np.zeros((128, 640), np.float32)
    for qq in range(4):
        for g in range(4):
            for jp in range(8):
                r = 32 * qq + 8 * g + jp
                allc2[K + jp, 128 * qq + r] = 1.0
    mask = np.zeros((128, 128), np.float32)
    for r in range(128):
        g = (r % 32) // 8
        mask[r, 32 * g:32 * g + 32] = 1.0
    allc2[:, 512:640] = mask
    in_maps = []
    for c in range(NC_):
        in_maps.append({
            "nf": nf[c * NLOC:(c + 1) * NLOC],
            "fragf": frag,
            "fragl": np.ascontiguousarray(frag[c * GPC * J:(c + 1) * GPC * J]),
            "allc": allc, "allc2": allc2,
        })
    return in_maps


def _node_perm():
    # column c of each graph's tiles holds node n(c); see module docstring.
    c = np.arange(NPG)
    return 256 * (c // 256) + 2 * (c % 128) + (c // 128) % 2


def kernel(**inputs):
    from concourse.bass_utils import run_bass_kernel_spmd

    nc = _get_program()
    in_maps = make_in_maps(inputs)
    res = run_bass_kernel_spmd(nc, in_maps, core_ids=list(range(NC_)))
    raw = np.concatenate([r["out"] for r in res.results], axis=0)
    out = np.empty_like(raw)
    out[:, _node_perm()] = 0.5 * raw + 0.5
    return out
